# revision 9
# baseline (speedup 1.0000x reference)
"""3x3 median filter (reflect padding) on Trainium2, 8-core data parallel.

Layout (per core, 4 images):
  partition p = b*32 + g
    b in 0..3  : image index within the core's batch shard
    g in 0..31 : group of 7 consecutive output rows
  linear(p) = p*7*W*C addresses (b,g) jointly (the strides nest
  perfectly), so one 3-dim access pattern spans all 128 partitions.

All device compute is fp16 (host converts): 2-byte packed operands put
the DVE in its 2x perf mode (measured 0.553 ns/elem sustained; fp32
and u8 both run 1x) and halve DMA traffic; max quantization error
~2.5e-4 against a 2e-2 tolerance.

A single 9-row slab per partition (rows 7g-1 .. 7g+7) is loaded once
in three sub-waves on the sync + scalar HW-DGE queues: A1 = slab rows
0-1, A2 = rows 2-5, B = rows 6-8.  Each sub-wave is one contiguous
multi-row packet per partition that over-reads DRAM to absorb the
halo; the image-boundary partitions are patched by small reflect DMAs
issued on the vector queue (top) and sync queue (bottom) so the first
compute op waits only on A1 + the top patches.  The A2 wave reaches
slab row 5, which makes the entire row 0-3 pipeline (stage 1, merge,
final med3, store) independent of wave B.

Median of 9 = med3( max3(col_lows), med3(col_meds), min3(col_highs) )
with each vertical column triple sorted once and shared across the
three horizontally adjacent windows.  The vertical sort additionally
shares the row-pair min/max between the two triples that contain the
pair (triple k = pair(k,k+1) + row k+2 for even k, row k + pair
(k+1,k+2)... realized as: triple k uses pair j = k for even k, j =
k+1 for odd k), cutting stage-1 volume from 6N to 5.14N at the price
of even/odd split combine ops (stride-2 row access patterns keep the
DVE in 2x mode, which only requires a packed last dim).

Horizontal neighbor access is a +-3 float shift inside each row; the
image's first/last output columns are recomputed exactly with narrow
per-column ops and overwritten before the store.  The final med3
chain + store go out in three row groups so stores overlap the tail
compute.
"""

import sys

if "/opt/trn_rl_repo" not in sys.path:
    sys.path.insert(0, "/opt/trn_rl_repo")

import numpy as np

import concourse.bass as bass  # noqa: F401
import concourse.tile as tile
from concourse import bacc, mybir
from concourse.ap import AP
from concourse.bass_utils import run_bass_kernel_spmd

F32 = mybir.dt.float32
F16 = mybir.dt.float16
MIN = mybir.AluOpType.min
MAX = mybir.AluOpType.max

B, H, W, C = 32, 224, 224, 3
NCORES = 8
BPC = B // NCORES      # 4 images per core
NG, GR = 32, 7         # row-groups per image, rows per group
WC = W * C             # 672 floats per image row
IMG = H * WC
PS = GR * WC           # 4704: per-partition linear stride
R = GR                 # 7 output rows per partition
N = R * WC             # 4704 output floats per partition
SRR = R + 2            # 9 slab rows

_CACHE = {}


def _build_kernel(tc, y, x):
    nc = tc.nc
    qa, qb, qv = nc.sync, nc.scalar, nc.gpsimd

    with tc.tile_pool(name="sb", bufs=1) as sb:
        S = sb.tile([128, SRR, WC], F16, tag="s", name="S")

        def rows(q, p0, p1, dram_row, s0, nr):
            q.dma_start(S[p0:p1, s0:s0 + nr, :],
                        AP(x.tensor, p0 * PS + dram_row * WC,
                           [[PS, p1 - p0], [1, nr * WC]]))

        # ---- loads ------------------------------------------------
        # gpsimd SWDGE: p0 slab row 1 (no WAW with any wave; lands in
        # parallel with A1)
        qv.dma_start(S[0:1, 1:2, :],
                     AP(x.tensor, 0, [[1, WC]]))
        # wave A1: slab rows 0..1 (dram -1..0, over-read for the halo)
        rows(qa, 1, 64, -1, 0, 2)
        rows(qb, 64, 128, -1, 0, 2)
        # reflect top patch (slab row 0 at p = 0,32,64,96): on the
        # otherwise-idle gpsimd queue, emitted AFTER A1 so the WAW
        # with A1's over-read resolves patch-last (the SWDGE issue
        # waits on A1's completion semaphores off the critical queues)
        qv.dma_start(S[0:128:32, 0:1, :],
                     AP(x.tensor, WC, [[IMG, 4], [1, WC]]))
        # wave A2: slab rows 2..5 (dram 1..4)
        rows(qa, 0, 64, 1, 2, 4)
        rows(qb, 64, 128, 1, 2, 4)
        # wave B: slab rows 6..8 (dram 5..7, over-read; p127 special)
        rows(qa, 0, 64, 5, 6, 3)
        rows(qb, 64, 127, 5, 6, 3)
        qb.dma_start(S[127:128, 6:8, :],     # p127 slab rows 6..7
                     AP(x.tensor, 127 * PS + 5 * WC, [[1, 2 * WC]]))
        # reflect bottom: slab row 8 at p = 31,63,95,127 (gpsimd,
        # emitted after wave B for the same WAW reason)
        qv.dma_start(S[31:128:32, 8:9, :],
                     AP(x.tensor, (H - 2) * WC, [[IMG, 4], [1, WC]]))

        # ---- stage 1: vertical column sort with pair sharing -------
        # Pp/Qp row j2 holds min/max of slab rows (2*j2, 2*j2+1).
        # Triple k (slab rows k..k+2) uses pair j = k (k even, third
        # row below) or j = k+1 (k odd, third row above).
        Pp = sb.tile([128, 4, WC], F16, tag="pp", name="Pp")
        Qp = sb.tile([128, 4, WC], F16, tag="qp", name="Qp")
        LO = sb.tile([128, R, WC], F16, tag="lo", name="LO")
        ME = sb.tile([128, R, WC], F16, tag="me", name="ME")
        HI = sb.tile([128, R, WC], F16, tag="hi", name="HI")
        T1 = sb.tile([128, R, WC], F16, tag="t1", name="T1")

        # pairs j=0 (needs A1 + top patches only)
        nc.vector.tensor_tensor(Pp[:, 0:1], S[:, 0:1], S[:, 1:2], MIN)
        nc.vector.tensor_tensor(Qp[:, 0:1], S[:, 0:1], S[:, 1:2], MAX)
        # pairs j=2,4 (needs A2)
        nc.vector.tensor_tensor(Pp[:, 1:3], S[:, 2:6:2], S[:, 3:6:2], MIN)
        nc.vector.tensor_tensor(Qp[:, 1:3], S[:, 2:6:2], S[:, 3:6:2], MAX)

        def combine(lo, me, hi, t1, pp, qp, c):
            nc.vector.tensor_tensor(lo, pp, c, MIN)
            nc.vector.tensor_tensor(t1, qp, c, MIN)
            nc.vector.tensor_tensor(hi, qp, c, MAX)
            nc.vector.tensor_tensor(me, pp, t1, MAX)

        # combines k=0,2 (pairs 0,1; c = S rows 2,4)
        combine(LO[:, 0:3:2], ME[:, 0:3:2], HI[:, 0:3:2], T1[:, 0:3:2],
                Pp[:, 0:2], Qp[:, 0:2], S[:, 2:5:2])
        # combines k=1,3 (pairs 1,2; c = S rows 1,3)
        combine(LO[:, 1:4:2], ME[:, 1:4:2], HI[:, 1:4:2], T1[:, 1:4:2],
                Pp[:, 1:3], Qp[:, 1:3], S[:, 1:4:2])

        M1 = sb.tile([128, R, WC], F16, tag="m1", name="M1")

        # ---- stage 2 declarations (bodies emitted in stream order) -
        E = WC - 3   # 669
        D = WC - 6   # 666
        U = sb.tile([128, R, WC], F16, tag="u", name="U")
        V = sb.tile([128, R, WC], F16, tag="v", name="V")
        Sm = sb.tile([128, R, WC], F16, tag="sm", name="Sm")
        Tm = sb.tile([128, R, WC], F16, tag="tm", name="Tm")
        MT = sb.tile([128, R, WC], F16, tag="mt", name="MT")
        A = U   # max3 of lows
        Cc = V  # min3 of highs
        Bm = Sm  # med3 of meds

        def merge(ra, rb):
            nc.vector.tensor_tensor(U[:, ra:rb, 0:E], LO[:, ra:rb, 0:E],
                                    LO[:, ra:rb, 3:WC], MAX)
            nc.vector.tensor_tensor(U[:, ra:rb, 0:D], U[:, ra:rb, 0:D],
                                    LO[:, ra:rb, 6:WC], MAX)
            nc.vector.tensor_tensor(V[:, ra:rb, 0:E], HI[:, ra:rb, 0:E],
                                    HI[:, ra:rb, 3:WC], MIN)
            nc.vector.tensor_tensor(V[:, ra:rb, 0:D], V[:, ra:rb, 0:D],
                                    HI[:, ra:rb, 6:WC], MIN)
            nc.vector.tensor_tensor(Sm[:, ra:rb, 0:E], ME[:, ra:rb, 0:E],
                                    ME[:, ra:rb, 3:WC], MIN)
            nc.vector.tensor_tensor(Tm[:, ra:rb, 0:E], ME[:, ra:rb, 0:E],
                                    ME[:, ra:rb, 3:WC], MAX)
            nc.vector.tensor_tensor(Tm[:, ra:rb, 0:D], Tm[:, ra:rb, 0:D],
                                    ME[:, ra:rb, 6:WC], MIN)
            nc.vector.tensor_tensor(Sm[:, ra:rb, 0:D], Sm[:, ra:rb, 0:D],
                                    Tm[:, ra:rb, 0:D], MAX)

        def final_compute(ra, rb):
            nc.vector.tensor_tensor(MT[:, ra:rb, 0:D], A[:, ra:rb, 0:D],
                                    Bm[:, ra:rb, 0:D], MIN)
            nc.vector.tensor_tensor(A[:, ra:rb, 0:D], A[:, ra:rb, 0:D],
                                    Bm[:, ra:rb, 0:D], MAX)
            nc.vector.tensor_tensor(Cc[:, ra:rb, 0:D], A[:, ra:rb, 0:D],
                                    Cc[:, ra:rb, 0:D], MIN)
            nc.vector.tensor_tensor(M1[:, ra:rb, 3:WC - 3],
                                    MT[:, ra:rb, 0:D],
                                    Cc[:, ra:rb, 0:D], MAX)

        def store(ra, rb):
            for (p0, p1, q) in ((0, 64, qa), (64, 128, qb)):
                dst = AP(y.tensor, p0 * PS + ra * WC,
                         [[PS, p1 - p0], [WC, rb - ra], [1, WC]])
                q.dma_start(dst, M1[p0:p1, ra:rb, :])

        # pairs j=6 (needs B)
        nc.vector.tensor_tensor(Pp[:, 3:4], S[:, 6:7], S[:, 7:8], MIN)
        nc.vector.tensor_tensor(Qp[:, 3:4], S[:, 6:7], S[:, 7:8], MAX)
        # combines k=4,6 (pairs 2,3; c = S rows 6,8)
        combine(LO[:, 4:7:2], ME[:, 4:7:2], HI[:, 4:7:2], T1[:, 4:7:2],
                Pp[:, 2:4], Qp[:, 2:4], S[:, 6:9:2])
        # combines k=5 (pair 3; c = S row 5)
        combine(LO[:, 5:6], ME[:, 5:6], HI[:, 5:6], T1[:, 5:6],
                Pp[:, 3:4], Qp[:, 3:4], S[:, 5:6])

        # ---- exact first/last output columns (reflect), both at once
        # col 0: window cols (1,0,1) -> med3(max(lo0,lo1), med1,
        # min(hi0,hi1)); col 223: window cols (222,223,222).
        L4 = LO.rearrange("p r (a c) -> p r a c", a=W, c=C)
        H4 = HI.rearrange("p r (a c) -> p r a c", a=W, c=C)
        T4 = ME.rearrange("p r (a c) -> p r a c", a=W, c=C)
        M4 = M1.rearrange("p r (a c) -> p r a c", a=W, c=C)
        lo_o = L4[:, :, 0:W:W - 1, :]      # cols {0, 223}
        lo_i = L4[:, :, 1:W:W - 3, :]      # cols {1, 222}
        hi_o = H4[:, :, 0:W:W - 1, :]
        hi_i = H4[:, :, 1:W:W - 3, :]
        be = T4[:, :, 1:W:W - 3, :]        # med of inner col
        ae = sb.tile([128, R, 2, C], F16, tag="ae", name="ae")
        ce = sb.tile([128, R, 2, C], F16, tag="ce", name="ce")
        mem = sb.tile([128, R, 2, C], F16, tag="mm", name="mm")
        nc.vector.tensor_tensor(ae[:], lo_o, lo_i, MAX)
        nc.vector.tensor_tensor(ce[:], hi_o, hi_i, MIN)
        nc.vector.tensor_tensor(mem[:], ae[:], be, MIN)
        nc.vector.tensor_tensor(ae[:], ae[:], be, MAX)
        nc.vector.tensor_tensor(ce[:], ae[:], ce[:], MIN)
        nc.vector.tensor_tensor(M4[:, :, 0:W:W - 1, :], mem[:], ce[:], MAX)

        # ---- tail: single merge pass over all rows (wave B lands
        # well before stage-1 A-work drains, so no stall), then the
        # final med3 in three groups with a 1-row last group so the
        # last store is minimal
        merge(0, 7)
        final_compute(0, 4)
        store(0, 4)
        final_compute(4, 6)
        store(4, 6)
        final_compute(6, 7)
        store(6, 7)


def _build():
    if "nc" in _CACHE:
        return _CACHE["nc"]
    nc = bacc.Bacc("TRN2", target_bir_lowering=False, debug=False)
    x = nc.dram_tensor("x", [BPC, H, W, C], F16, kind="ExternalInput").ap()
    y = nc.dram_tensor("y", [BPC, H, W, C], F16, kind="ExternalOutput").ap()
    with tile.TileContext(nc) as tc:
        _build_kernel(tc, y, x)
    nc.compile()
    _CACHE["nc"] = nc
    return nc


def run(input_batch, **spmd_kwargs):
    nc = _build()
    xh = np.ascontiguousarray(input_batch).astype(np.float16)
    in_maps = [
        {"x": np.ascontiguousarray(xh[i * BPC:(i + 1) * BPC])}
        for i in range(NCORES)
    ]
    res = run_bass_kernel_spmd(nc, in_maps, list(range(NCORES)), **spmd_kwargs)
    out = np.concatenate([r["y"] for r in res.results],
                         axis=0).astype(np.float32)
    return out, res


def kernel(input_batch):
    out, _ = run(np.asarray(input_batch))
    return out
